# revision 1
# baseline (speedup 1.0000x reference)
"""MetricPatchEmbed Trainium kernel: 8-way data-parallel over batch."""
import sys

sys.path.insert(0, "/opt/trn_rl_repo")

import numpy as np
import concourse.bass as bass
import concourse.tile as tile
from concourse import mybir
from concourse import bass_utils

# ---- tile_patch: this walrus build allows only 1 sem wait per instruction ----
from concourse._compat import not_none as nn

_MAX_WAITS = 1


def _patched_drain_and_barrier(self, tick_clock, wait_clock):
    ScopedClock = tile.ScopedClock
    drain_inst = self.nc.sync.drain()
    wait_clock.add_sem_waits(
        drain_inst.ins, ScopedClock({None: tick_clock.global_clock})
    )
    si = drain_inst.ins.sync_info
    if si is not None and si.on_wait and len(si.on_wait) > _MAX_WAITS:
        waits = list(si.on_wait)
        cur_bb = nn(self.nc.cur_bb).bb
        spill = []
        while len(waits) > _MAX_WAITS:
            chunk, waits = waits[:_MAX_WAITS], waits[_MAX_WAITS:]
            nop = self.nc.sync.nop(nofuse=True, hint="drain_wait_spill")
            nop.ins.sync_info = mybir.SyncInfo(on_wait=chunk, on_update=[])
            spill.append(nop.ins)
        si.on_wait = waits
        insts = cur_bb.instructions
        spill_names = {i.name for i in spill}
        drain_pos = next(
            i for i, ins in enumerate(insts) if ins.name == drain_inst.ins.name
        )
        rest = [ins for ins in insts if ins.name not in spill_names]
        cur_bb.instructions = rest[:drain_pos] + spill + rest[drain_pos:]

    self.nc.all_engine_barrier()
    assert self.sems is not None
    popped = self.nc._tile_sem_poison_stack.pop()
    assert popped is self._sem_poison
    self.nc.clear_and_free_semaphores(list(self.sems.allocated().values()))
    self.nc.all_engine_barrier()
    _split_multi_waits(self.nc)


def _split_multi_waits(nc):
    for f in nc.m.functions:
        for bb in f.blocks:
            insts = bb.instructions
            changed = False
            out = []
            for ins in insts:
                si = ins.sync_info
                if si is not None and si.on_wait and len(si.on_wait) > _MAX_WAITS:
                    waits = list(si.on_wait)
                    while len(waits) > _MAX_WAITS:
                        chunk, waits = waits[:_MAX_WAITS], waits[_MAX_WAITS:]
                        nop = mybir.InstNoOp(
                            name=f"I-{nc.next_id()}-waitspill", ins=[], outs=[]
                        )
                        nop.engine = ins.engine
                        nop.sync_info = mybir.SyncInfo(on_wait=chunk, on_update=[])
                        out.append(nop)
                        changed = True
                    si.on_wait = waits
                out.append(ins)
            if changed:
                bb.instructions = out


tile.TileContext._drain_and_barrier = _patched_drain_and_barrier

# ---------------- problem constants ----------------
IMG = 112
P = 7
R = 3
V = 256
C = 768
EPS_W = 0.5
B = 8
HP = IMG // P          # 16
NPATCH = HP * HP       # 256
NK = P * P             # 49
K = NK * V             # 12544 contraction dim, order (kykx, v)
NKT = K // 128         # 98 k-tiles
W_MARGIN = 8           # window half-width beyond patch
S = P + 2 * W_MARGIN   # 23 window side
SW = S * S             # 529
SWPAD = SW + S + 2     # padded for +S+1 shifts

F32 = mybir.dt.float32
BF16 = mybir.dt.bfloat16
AL = mybir.AluOpType
AF = mybir.ActivationFunctionType

_CACHE = {}


def _consts():
    if "c" in _CACHE:
        return _CACHE["c"]
    gy, gx = np.meshgrid(np.arange(-R, R + 1, dtype=np.float32),
                         np.arange(-R, R + 1, dtype=np.float32), indexing="ij")
    g = np.stack([gy, gx], -1).reshape(NK, 2)
    nrm = np.maximum(np.linalg.norm(g, axis=-1, keepdims=True), 1e-12)
    uhat = g / nrm
    ii, jj = np.meshgrid(np.arange(HP), np.arange(HP), indexing="ij")
    cy = (ii * P + R).astype(np.float32).reshape(NPATCH)      # patch center y
    cx = (jj * P + R).astype(np.float32).reshape(NPATCH)
    _CACHE["c"] = (g, uhat, cy, cx)
    return _CACHE["c"]


def _host_prep(x, metric_w, metric_b, proj_w, proj_b):
    """Static relayouts (no data-dependent compute)."""
    g, uhat, cy, cx = _consts()
    x = np.asarray(x).astype(np.int32)
    imgs = x.reshape(B, IMG, IMG)

    # imgp: value at patch-pixel, laid out (kykx, patch) kykx-major
    # pixel for (patch (i,j), (ky,kx)) = (7i+ky, 7j+kx)
    t = imgs.reshape(B, HP, P, HP, P).transpose(0, 2, 4, 1, 3)  # B,ky,kx,i,j
    imgp = t.reshape(B, NK * NPATCH).astype(np.float32)         # (kykx, patch)

    # window image per patch: rows [cy-11 .. cy+11], cols [cx-11 .. cx+11]
    pad = np.zeros((B, IMG + 2 * W_MARGIN + S, IMG + 2 * W_MARGIN + S), np.float32)
    pad[:, W_MARGIN:W_MARGIN + IMG, W_MARGIN:W_MARGIN + IMG] = imgs
    win = np.zeros((B, NPATCH, SWPAD), np.float32)
    for pi in range(HP):
        for pj in range(HP):
            p_ = pi * HP + pj
            blk = pad[:, pi * P:pi * P + S, pj * P:pj * P + S].reshape(B, SW)
            win[:, p_, :SW] = blk
    # window top-left in image coords: (cy - 11, cx - 11)
    base = ((cy - (R + W_MARGIN)) * S + (cx - (R + W_MARGIN))).astype(np.float32)

    wm = np.asarray(metric_w, np.float32)   # (7, V, P, P)
    wp = np.asarray(proj_w, np.float32)     # (C, V, P, P)
    # K order: k = (ky*P+kx)*V + v
    wmr = wm.transpose(2, 3, 1, 0).reshape(K, 7).astype(np.float32)
    wpr = wp.transpose(2, 3, 1, 0).reshape(K, C)
    import ml_dtypes
    wmr_bf = wmr.astype(ml_dtypes.bfloat16)
    wpr_bf = wpr.astype(ml_dtypes.bfloat16)

    # per-kykx constants replicated across 128 partitions
    uy = uhat[:, 0].astype(np.float32)
    ux = uhat[:, 1].astype(np.float32)
    kconst = np.stack([uy * uy, 2 * uy * ux, ux * ux, uy, ux,
                       g[:, 0], g[:, 1]], 0)  # (7, NK)
    kconst_b = np.broadcast_to(kconst[None], (128, 7, NK)).reshape(128, 7 * NK).copy()

    # per-patch scalars, patch-major tiles (2 tiles of 128)
    pconst = np.stack([cy, cx, base], 0).astype(np.float32)  # (3, 256)

    ident7 = np.eye(7, dtype=np.float32)
    bias_m = np.broadcast_to(np.asarray(metric_b, np.float32)[None], (128, 7)).copy()
    bias_p = np.broadcast_to(np.asarray(proj_b, np.float32)[None], (128, C)).copy()
    return dict(imgp=imgp, win=win.reshape(B, NPATCH * SWPAD), wmr=wmr_bf,
                wpr=wpr_bf, kconst=kconst_b, pconst=pconst, ident7=ident7,
                bias_m=bias_m, bias_p=bias_p)


def _build():
    if "nc" in _CACHE:
        return _CACHE["nc"]
    nc = bass.Bass()
    d_imgp = nc.dram_tensor("imgp", [1, K], F32, kind="ExternalInput")
    d_win = nc.dram_tensor("win", [NPATCH, SWPAD], F32, kind="ExternalInput")
    d_wmr = nc.dram_tensor("wmr", [K, 7], BF16, kind="ExternalInput")
    d_wpr = nc.dram_tensor("wpr", [K, C], BF16, kind="ExternalInput")
    d_kc = nc.dram_tensor("kconst", [128, 7 * NK], F32, kind="ExternalInput")
    d_pc = nc.dram_tensor("pconst", [3, NPATCH], F32, kind="ExternalInput")
    d_id7 = nc.dram_tensor("ident7", [7, 7], F32, kind="ExternalInput")
    d_bm = nc.dram_tensor("bias_m", [128, 7], F32, kind="ExternalInput")
    d_bp = nc.dram_tensor("bias_p", [128, C], F32, kind="ExternalInput")
    d_out = nc.dram_tensor("out", [NPATCH, C], F32, kind="ExternalOutput")
    # scratch dram for A transpose round-trip
    d_apm = nc.dram_tensor("apm_scratch", [NPATCH, K], BF16, kind="Internal")

    def bcast(ap, n):
        return bass.AP(tensor=ap.tensor, offset=ap.offset,
                       ap=[[0, n]] + list(ap.ap[1:]))

    with tile.TileContext(nc) as tc:
        import contextlib
        with contextlib.ExitStack() as ctx:
            singles = ctx.enter_context(tc.tile_pool(name="singles", bufs=1))
            # iotas
            viota = singles.tile([128, V], F32)
            nc.gpsimd.iota(viota, pattern=[[1, V]], base=0, channel_multiplier=0,
                           allow_small_or_imprecise_dtypes=True)
            piota = singles.tile([128, 2], F32)
            nc.gpsimd.iota(piota, pattern=[[128, 2]], base=0, channel_multiplier=1,
                           allow_small_or_imprecise_dtypes=True)
            liota = singles.tile([128, SW], F32)
            nc.gpsimd.iota(liota, pattern=[[1, SW]], base=0, channel_multiplier=0,
                           allow_small_or_imprecise_dtypes=True)

            kc = singles.tile([128, 7 * NK], F32)
            nc.sync.dma_start(out=kc, in_=d_kc[:, :])
            id7 = singles.tile([7, 7], F32)
            nc.sync.dma_start(out=id7, in_=d_id7[:, :])
            bm = singles.tile([128, 7], F32)
            nc.sync.dma_start(out=bm, in_=d_bm[:, :])
            bp = singles.tile([128, C], F32)
            nc.sync.dma_start(out=bp, in_=d_bp[:, :])
            # per-patch consts -> [128, 3] per ptile via small strided dma
            pconsts = []
            for pt in range(2):
                t_ = singles.tile([128, 3], F32, name=f"pc{pt}", tag=f"pc{pt}")
                src = d_pc[:, pt * 128:(pt + 1) * 128]
                # transpose 3x128 -> 128x3 via AP (small, strided descriptors)
                ap = bass.AP(tensor=src.tensor, offset=src.offset,
                             ap=[[1, 128], [NPATCH, 3]])

                nc.sync.dma_start(out=t_, in_=ap)
                pconsts.append(t_)
            win_ts = []
            for pt in range(2):
                t_ = singles.tile([128, SWPAD], F32, name=f"win{pt}", tag=f"win{pt}")
                nc.sync.dma_start(out=t_, in_=d_win[pt * 128:(pt + 1) * 128, :])
                win_ts.append(t_)

            # ---- stage 1: params = metric conv via one-hot matmul ----
            params_ps = None
            with tc.tile_pool(name="s1", bufs=1) as s1, \
                 tc.tile_pool(name="s1ps", bufs=1, space="PSUM") as s1ps:
                imgb = s1.tile([128, K], F32)
                nc.sync.dma_start(out=imgb, in_=bcast(d_imgp[0:1, :], 128))
                wm_sb = s1.tile([128, NKT * 7], BF16)
                wmr_ap = d_wmr[:, :]
                nc.sync.dma_start(
                    out=wm_sb,
                    in_=bass.AP(tensor=wmr_ap.tensor, offset=wmr_ap.offset,
                                ap=[[7, 128], [7 * 128, NKT], [1, 7]]))
                oht = s1.tile([128, K // 128 * 256], BF16)  # 98 tiles x [128,256]
                # k-tile kt covers k = kt*128 ..: kykx = kt//2, vhalf = kt%2
                params_ps = s1ps.tile([7, NPATCH], F32)
                for kt in range(NKT):
                    kykx, vh = kt // 2, kt % 2
                    src = imgb[:, kykx * NPATCH:(kykx + 1) * NPATCH]
                    dst = oht[:, kt * 256:(kt + 1) * 256]
                    nc.vector.tensor_scalar(dst, src, piota[:, vh:vh + 1],
                                            None, AL.is_equal)
                    nc.tensor.matmul(params_ps, wm_sb[:, kt * 7:(kt + 1) * 7],
                                     dst, start=(kt == 0), stop=(kt == NKT - 1))
                # transpose params -> patch major [128,7] x2
                par_sb7 = s1.tile([7, NPATCH], F32)
                nc.scalar.copy(par_sb7, params_ps)
                par = []
                for pt in range(2):
                    tps = s1ps.tile([128, 7], F32)
                    nc.tensor.transpose(tps, par_sb7[:, pt * 128:(pt + 1) * 128],
                                        id7)
                    sb = singles.tile([128, 7], F32, name=f"par{pt}", tag=f"par{pt}")
                    nc.scalar.copy(sb, tps)
                    nc.vector.tensor_tensor(sb, sb, bm, AL.add)
                    par.append(sb)

            # ---- stage 2+3 per patch-tile ----
            apm_tiles = []
            with tc.tile_pool(name="s2", bufs=1) as s2:
                for pt in range(2):
                    p_ = par[pt]
                    cy_s = pconsts[pt][:, 0:1]
                    cx_s = pconsts[pt][:, 1:2]
                    base_s = pconsts[pt][:, 2:3]
                    t = s2.tile([128, 16], F32, tag="t")
                    # metric vector v
                    nc.vector.tensor_tensor(t[:, 0:2], p_[:, 0:2], p_[:, 0:2], AL.mult)
                    nc.vector.tensor_tensor(t[:, 2:3], t[:, 0:1], t[:, 1:2], AL.add)
                    nc.scalar.activation(t[:, 3:4], t[:, 2:3], AF.Sqrt)
                    nc.vector.tensor_scalar_max(t[:, 3:4], t[:, 3:4], 1e-12)
                    nc.vector.reciprocal(t[:, 4:5], t[:, 3:4])
                    v0 = t[:, 5:6]; v1 = t[:, 6:7]
                    nc.vector.tensor_scalar(v0, p_[:, 0:1], t[:, 4:5], None, AL.mult)
                    nc.vector.tensor_scalar(v1, p_[:, 1:2], t[:, 4:5], None, AL.mult)
                    sg = s2.tile([128, 3], F32, tag="sg")
                    nc.scalar.activation(sg, p_[:, 2:5], AF.Sigmoid)
                    sc = t[:, 7:8]
                    nc.vector.tensor_scalar(sc, sg[:, 2:3], 1.5, 0.5, AL.mult, AL.add)
                    e1 = t[:, 8:9]; e2 = t[:, 9:10]
                    nc.vector.tensor_scalar(e1, sg[:, 0:1], 2.0, None, AL.mult)
                    nc.vector.tensor_scalar(e1, e1, sc, None, AL.mult)
                    nc.vector.tensor_scalar(e2, sg[:, 1:2], 2.0, None, AL.mult)
                    nc.vector.tensor_scalar(e2, e2, sc, None, AL.mult)
                    # M entries
                    m = s2.tile([128, 3], F32, tag="m")  # m11, m12x2?, m22
                    q = s2.tile([128, 4], F32, tag="q")
                    nc.vector.tensor_tensor(q[:, 0:1], v0, v0, AL.mult)
                    nc.vector.tensor_tensor(q[:, 1:2], v1, v1, AL.mult)
                    nc.vector.tensor_tensor(q[:, 2:3], v0, v1, AL.mult)
                    nc.vector.tensor_tensor(q[:, 3:4], e1, e2, AL.subtract)
                    nc.vector.tensor_scalar(m[:, 0:1], q[:, 0:1], e1, None, AL.mult)
                    nc.vector.scalar_tensor_tensor(m[:, 0:1], q[:, 1:2], e2,
                                                   m[:, 0:1], AL.mult, AL.add)
                    nc.vector.tensor_tensor(m[:, 1:2], q[:, 2:3], q[:, 3:4], AL.mult)
                    nc.vector.tensor_scalar(m[:, 2:3], q[:, 1:2], e1, None, AL.mult)
                    nc.vector.scalar_tensor_tensor(m[:, 2:3], q[:, 0:1], e2,
                                                   m[:, 2:3], AL.mult, AL.add)
                    # drift w
                    wd = s2.tile([128, 4], F32, tag="wd")
                    nc.vector.tensor_tensor(wd[:, 0:2], p_[:, 5:7], p_[:, 5:7], AL.mult)
                    nc.vector.tensor_tensor(wd[:, 2:3], wd[:, 0:1], wd[:, 1:2], AL.add)
                    nc.scalar.activation(wd[:, 3:4], wd[:, 2:3], AF.Sqrt)
                    nc.scalar.activation(wd[:, 3:4], wd[:, 3:4], AF.Sigmoid)
                    nc.vector.tensor_scalar(wd[:, 3:4], wd[:, 3:4], 1.0 - EPS_W,
                                            None, AL.mult)
                    wy = wd[:, 0:1]; wx = wd[:, 1:2]
                    nc.vector.tensor_scalar(wy, p_[:, 5:6], wd[:, 3:4], None, AL.mult)
                    nc.vector.tensor_scalar(wx, p_[:, 6:7], wd[:, 3:4], None, AL.mult)

                    # quad/Fr/y over [128, NK]
                    uy2 = kc[:, 0 * NK:1 * NK]; uyux2 = kc[:, 1 * NK:2 * NK]
                    ux2 = kc[:, 2 * NK:3 * NK]; uyc = kc[:, 3 * NK:4 * NK]
                    uxc = kc[:, 4 * NK:5 * NK]; gyc = kc[:, 5 * NK:6 * NK]
                    gxc = kc[:, 6 * NK:7 * NK]
                    fr = s2.tile([128, NK], F32, tag="fr")
                    nc.vector.tensor_scalar(fr, uy2, m[:, 0:1], None, AL.mult)
                    nc.vector.scalar_tensor_tensor(fr, uyux2, m[:, 1:2], fr,
                                                   AL.mult, AL.add)
                    nc.vector.scalar_tensor_tensor(fr, ux2, m[:, 2:3], fr,
                                                   AL.mult, AL.add)
                    nc.vector.tensor_scalar_max(fr, fr, 1e-12)
                    nc.scalar.activation(fr, fr, AF.Sqrt)
                    nc.vector.scalar_tensor_tensor(fr, uyc, wy, fr, AL.mult, AL.add)
                    nc.vector.scalar_tensor_tensor(fr, uxc, wx, fr, AL.mult, AL.add)
                    nc.vector.tensor_scalar_max(fr, fr, 1e-3)
                    nc.vector.reciprocal(fr, fr)
                    py = s2.tile([128, NK], F32, tag="py")
                    px = s2.tile([128, NK], F32, tag="px")
                    nc.vector.tensor_tensor(py, gyc, fr, AL.mult)
                    nc.vector.tensor_scalar(py, py, cy_s, None, AL.add)
                    nc.vector.tensor_tensor(px, gxc, fr, AL.mult)
                    nc.vector.tensor_scalar(px, px, cx_s, None, AL.add)
                    # floor via round(x - 0.5) int cast
                    y0 = s2.tile([128, NK], F32, tag="y0")
                    x0 = s2.tile([128, NK], F32, tag="x0")
                    iy = s2.tile([128, NK], mybir.dt.int32, tag="iy")
                    nc.vector.tensor_scalar(y0, py, 0.5, None, AL.subtract)
                    nc.vector.tensor_copy(iy, y0)
                    nc.vector.tensor_copy(y0, iy)
                    nc.vector.tensor_scalar(x0, px, 0.5, None, AL.subtract)
                    nc.vector.tensor_copy(iy, x0)
                    nc.vector.tensor_copy(x0, iy)
                    wyf = s2.tile([128, NK], F32, tag="wyf")
                    wxf = s2.tile([128, NK], F32, tag="wxf")
                    nc.vector.tensor_tensor(wyf, py, y0, AL.subtract)
                    nc.vector.tensor_tensor(wxf, px, x0, AL.subtract)
                    # validity: vy0=(0<=y0<=111), vy1=(0<=y0+1<=111) etc
                    vmask = s2.tile([128, 4 * NK], F32, tag="vmask")

                    tmpv = s2.tile([128, NK], F32, tag="tmpv")
                    for ci, (dy, dx) in enumerate(((0, 0), (0, 1), (1, 0), (1, 1))):
                        dstm = vmask[:, ci * NK:(ci + 1) * NK]
                        nc.vector.tensor_scalar(dstm, y0, float(-dy), None, AL.is_ge)
                        nc.vector.tensor_scalar(tmpv, y0, float(111 - dy), None,
                                                AL.is_le)
                        nc.vector.tensor_tensor(dstm, dstm, tmpv, AL.mult)
                        nc.vector.tensor_scalar(tmpv, x0, float(-dx), None, AL.is_ge)
                        nc.vector.tensor_tensor(dstm, dstm, tmpv, AL.mult)
                        nc.vector.tensor_scalar(tmpv, x0, float(111 - dx), None,
                                                AL.is_le)
                        nc.vector.tensor_tensor(dstm, dstm, tmpv, AL.mult)
                    # bilinear weights * validity
                    wgt = s2.tile([128, 4 * NK], F32, tag="wgt")
                    one_wy = s2.tile([128, NK], F32, tag="one_wy")
                    one_wx = s2.tile([128, NK], F32, tag="one_wx")
                    nc.vector.tensor_scalar(one_wy, wyf, 1.0, -1.0, AL.subtract,
                                            AL.mult)  # (wyf-1)*-1 = 1-wyf
                    nc.vector.tensor_scalar(one_wx, wxf, 1.0, -1.0, AL.subtract,
                                            AL.mult)
                    nc.vector.tensor_tensor(wgt[:, 0:NK], one_wy, one_wx, AL.mult)
                    nc.vector.tensor_tensor(wgt[:, NK:2 * NK], one_wy, wxf, AL.mult)
                    nc.vector.tensor_tensor(wgt[:, 2 * NK:3 * NK], wyf, one_wx, AL.mult)
                    nc.vector.tensor_tensor(wgt[:, 3 * NK:4 * NK], wyf, wxf, AL.mult)
                    nc.vector.tensor_tensor(wgt, wgt, vmask, AL.mult)
                    # lambda00 = y0*S + x0 - base
                    lam = s2.tile([128, NK], F32, tag="lam")
                    nc.vector.tensor_scalar(lam, y0, float(S), None, AL.mult)
                    nc.vector.tensor_tensor(lam, lam, x0, AL.add)
                    nc.vector.tensor_scalar(lam, lam, base_s, None, AL.subtract)
                    # vc extraction via window compare
                    vc = s2.tile([128, 4 * NK], F32, tag="vc")
                    eqm = s2.tile([128, SW], F32, tag="eqm")
                    junk = s2.tile([128, SW], F32, tag="junk")
                    wint = win_ts[pt]
                    for j in range(NK):
                        nc.vector.tensor_scalar(eqm, liota, lam[:, j:j + 1], None,
                                                AL.is_equal)
                        for ci, sh in enumerate((0, 1, S, S + 1)):
                            nc.vector.scalar_tensor_tensor(
                                junk, eqm, 1.0, wint[:, sh:sh + SW],
                                AL.mult, AL.mult,
                                accum_out=vc[:, ci * NK + j:ci * NK + j + 1])
                    # A build patch-major [128, K] bf16
                    apm = s2.tile([128, K], BF16, tag="apm")
                    tmpc = s2.tile([128, V], F32, tag="tmpc")
                    tmpd = s2.tile([128, V], F32, tag="tmpd")
                    for kykx in range(NK):
                        dst = apm[:, kykx * V:(kykx + 1) * V]
                        nc.vector.tensor_scalar(tmpc, viota,
                                                vc[:, 0 * NK + kykx:0 * NK + kykx + 1],
                                                wgt[:, 0 * NK + kykx:0 * NK + kykx + 1],
                                                AL.is_equal, AL.mult)
                        for ci in range(1, 4):
                            nc.vector.tensor_scalar(
                                tmpd, viota,
                                vc[:, ci * NK + kykx:ci * NK + kykx + 1],
                                wgt[:, ci * NK + kykx:ci * NK + kykx + 1],
                                AL.is_equal, AL.mult)
                            nc.vector.tensor_tensor(tmpc, tmpc, tmpd, AL.add)
                        nc.vector.tensor_copy(dst, tmpc)
                    nc.sync.dma_start(out=d_apm[pt * 128:(pt + 1) * 128, :], in_=apm)
                    apm_tiles.append(apm)

            # ---- stage 4: GEMM out = A @ Wp via dma-transposed A ----
            with tc.tile_pool(name="g", bufs=3) as gp, \
                 tc.tile_pool(name="gps", bufs=2, space="PSUM") as gps:
                at_t = None
                psums = []
                for pt in range(2):
                    for nh in range(2):
                        psums.append(gps.tile([128, C // 2], F32, name=f"ps{pt}{nh}", tag=f"ps{pt}{nh}"))
                for kt in range(NKT):
                    at_t = gp.tile([128, NPATCH], BF16, tag="at")
                    nc.sync.dma_start(
                        out=at_t, in_=d_apm[:, kt * 128:(kt + 1) * 128],
                        transpose=True)
                    wp_t = gp.tile([128, C], BF16, tag="wp")
                    nc.sync.dma_start(out=wp_t,
                                      in_=d_wpr[kt * 128:(kt + 1) * 128, :])
                    for pt in range(2):
                        for nh in range(2):
                            nc.tensor.matmul(
                                psums[pt * 2 + nh],
                                at_t[:, pt * 128:(pt + 1) * 128],
                                wp_t[:, nh * (C // 2):(nh + 1) * (C // 2)],
                                start=(kt == 0), stop=(kt == NKT - 1))
                for pt in range(2):
                    ot = gp.tile([128, C], F32, tag="ot")
                    for nh in range(2):
                        nc.scalar.copy(ot[:, nh * (C // 2):(nh + 1) * (C // 2)],
                                       psums[pt * 2 + nh])
                    nc.vector.tensor_tensor(ot, ot, bp, AL.add)
                    nc.sync.dma_start(out=d_out[pt * 128:(pt + 1) * 128, :], in_=ot)

    _CACHE["nc"] = nc
    return nc


def kernel(x, metric_w, metric_b, proj_w, proj_b):
    prep = _host_prep(x, metric_w, metric_b, proj_w, proj_b)
    nc = _build()
    in_maps = []
    for b in range(B):
        in_maps.append({
            "imgp": prep["imgp"][b:b + 1],
            "win": prep["win"][b].reshape(NPATCH, SWPAD),
            "wmr": prep["wmr"],
            "wpr": prep["wpr"],
            "kconst": prep["kconst"],
            "pconst": prep["pconst"],
            "ident7": prep["ident7"],
            "bias_m": prep["bias_m"],
            "bias_p": prep["bias_p"],
        })
    res = bass_utils.run_bass_kernel_spmd(nc, in_maps, core_ids=list(range(B)))
    out = np.stack([res.results[b]["out"] for b in range(B)], 0)
    return out.astype(np.float32)

